# revision 9
# baseline (speedup 1.0000x reference)
"""Two-layer GAT (heads=1) on Trainium2, 8 NeuronCores.

Sharding: destination-sharded graph parallel. Each core owns N/8 dst nodes
(dealt by global degree rank so all 8 cores share one SPMD program
structure). Node tables (attention projections + 16-wide features) are
AllGathered; per-edge feature rows are fetched with dma_gather (int16
indices, so the global slot space is split into 4 contiguous regions = core
pairs). Per region each core lays its edges on a [128 x cols] grid — one dst
per (partition, block), block width = max degree in the block with dsts
sorted by per-region degree — so the per-dst softmax and weighted
aggregation are free-axis reductions. exp() is applied without
max-subtraction (scores are O(10); safe in f32; matches the reference up to
rounding). Region partials are merged into the region-0 arrangement via
small dma_gathers over an HBM staging table. Layer 2 aggregates 16-wide h2
rows and applies W2 after aggregation ((sum a_ij h2_j) W2). log_softmax
runs on-chip; the host only reorders rows of the final output.
"""
import sys
import os

for _p in ("/opt/trn_rl_repo", "/root/.axon_site/_ro/trn_rl_repo"):
    if os.path.isdir(_p) and _p not in sys.path:
        sys.path.insert(0, _p)

import numpy as np

P = 128
NI = 1024          # indices per dma_gather call (single-packet limit)
TW = NI // P       # 8 grid columns per gather call
EB = 64            # table row width in f32 (256B gather element)
NCORES = 8
NREG = 4
NEG_SLOPE = 0.2


def _cdiv(a, b):
    return (a + b - 1) // b


# ------------------------------------------------------------------ host prep

def _prep(x, ei, W1, a_src1, a_dst1, b1, W2, a_src2, a_dst2, b2):
    N, DIM = x.shape
    HID = W1.shape[1]
    NCLS = W2.shape[1]
    assert N % NCORES == 0 and DIM % P == 0
    DPC = N // NCORES
    R = _cdiv(DPC, P)
    SLOTS = R * P
    NPAD = NCORES * SLOTS
    QSIZE = NPAD // NREG
    assert QSIZE < 32768 and SLOTS < 32768

    src = np.concatenate([np.asarray(ei[0]), np.arange(N)]).astype(np.int64)
    dst = np.concatenate([np.asarray(ei[1]), np.arange(N)]).astype(np.int64)

    deg = np.bincount(dst, minlength=N)
    order_global = np.argsort(-deg, kind="stable")
    core_of = np.empty(N, np.int64)
    core_of[order_global] = np.arange(N) % NCORES

    q_of_edge = core_of[src] // 2          # region = pair of cores holding src
    ecore = core_of[dst]

    deg_q = np.zeros((N, NREG), np.int64)
    np.add.at(deg_q, (dst, q_of_edge), 1)

    nodes_c = [order_global[np.arange(c, N, NCORES)] for c in range(NCORES)]

    rankq = [[None] * NREG for _ in range(NCORES)]   # padded-local-id -> rank
    lslot = np.full(N, -1, np.int64)
    gslot = np.full(N, -1, np.int64)
    sorted_degq = np.zeros((NCORES, NREG, SLOTS), np.int64)
    for c in range(NCORES):
        nds = nodes_c[c]
        for q in range(NREG):
            dq = np.zeros(SLOTS, np.int64)
            dq[: len(nds)] = deg_q[nds, q]
            o = np.argsort(-dq, kind="stable")
            rq = np.empty(SLOTS, np.int64)
            rq[o] = np.arange(SLOTS)
            rankq[c][q] = rq
            sorted_degq[c, q] = dq[o]
        lslot[nds] = rankq[c][0][: len(nds)]
        gslot[nds] = SLOTS * c + lslot[nds]

    # shared block widths per region
    D, offs, CQ = [], [], []
    for q in range(NREG):
        Dq = sorted_degq[:, q, ::P].max(axis=0)
        Bq = max(int(np.count_nonzero(Dq)), 1)
        Dq = Dq[:Bq].astype(np.int64)
        Dq = np.maximum(Dq, 1)                        # keep blocks non-empty
        off = np.concatenate([[0], np.cumsum(Dq)])
        Cq_pad = _cdiv(int(off[-1]), TW) * TW
        D.append(Dq)
        offs.append(off)
        CQ.append(Cq_pad)

    CT = sum(CQ)
    calls_main = CT // TW
    auxcalls = _cdiv(SLOTS, NI)

    # per-call (c0, w, b) ranges, shared across cores
    tiles = []
    for q in range(NREG):
        Dq, off = D[q], offs[q]
        Bq = len(Dq)
        bounds = off[1:]
        reg_tiles = []
        for k in range(CQ[q] // TW):
            c0g = k * TW
            ranges = []
            col = c0g
            while col < c0g + TW:
                b = int(np.searchsorted(bounds, col, side="right"))
                if b >= Bq:                            # padding cols at tail
                    ranges.append((col - c0g, c0g + TW - col, Bq - 1))
                    break
                end = min(int(bounds[b]), c0g + TW)
                ranges.append((col - c0g, end - col, b))
                col = end
            reg_tiles.append(ranges)
        tiles.append(reg_tiles)

    meta = dict(N=N, DIM=DIM, HID=HID, NCLS=NCLS, DPC=DPC, R=R, SLOTS=SLOTS,
                NPAD=NPAD, QSIZE=QSIZE, D=D, offs=offs, CQ=CQ, CT=CT,
                calls_main=calls_main, auxcalls=auxcalls, tiles=tiles)

    def wrap_grid(grid):
        """[128, C] slot grid -> wrapped idx stream [128, 64*(C//TW)] int16."""
        ncall = grid.shape[1] // TW
        g = grid.reshape(P, ncall, TW)
        v = g.transpose(1, 2, 0).reshape(ncall, NI)        # slot i = c*128+p
        w = v.reshape(ncall, NI // 16, 16).transpose(0, 2, 1)
        w16 = w.transpose(1, 0, 2).reshape(16, ncall * 64)
        return np.tile(w16, (8, 1)).astype(np.int16)

    def wrap_flat(v):
        ncall = len(v) // NI
        w = v.reshape(ncall, NI // 16, 16).transpose(0, 2, 1)
        w16 = w.transpose(1, 0, 2).reshape(16, ncall * 64)
        return np.tile(w16, (8, 1)).astype(np.int16)

    in_maps = []
    for c in range(NCORES):
        nds = nodes_c[c]
        loc = np.full(N, -1, np.int64)
        loc[nds] = np.arange(len(nds))
        grid = np.zeros((P, CT), np.int64)
        mask = np.full((P, CT), -30.0, np.float32)
        colbase = 0
        for q in range(NREG):
            sel = (ecore == c) & (q_of_edge == q)
            s_e = src[sel]
            d_e = dst[sel]
            rk = rankq[c][q][loc[d_e]]
            o = np.argsort(rk, kind="stable")
            rs = rk[o]
            if len(rs):
                newgrp = np.concatenate([[True], rs[1:] != rs[:-1]])
                gidx = np.cumsum(newgrp) - 1
                grp_start = np.flatnonzero(newgrp)
                cum = np.arange(len(rs)) - grp_start[gidx]
                p_e = rs % P
                b_e = rs // P
                col = offs[q][b_e] + cum + colbase
                grid[p_e, col] = gslot[s_e[o]] - QSIZE * q
                mask[p_e, col] = 0.0
            colbase += CQ[q]
        assert grid.min() >= 0 and grid.max() < 32768

        ads, cmbs = [], []
        for q in range(1, NREG):
            rq = rankq[c][q]
            inv = np.empty(SLOTS, np.int64)
            inv[rq] = np.arange(SLOTS)                # rank -> padded-local
            l0 = rankq[c][0]
            inv0 = np.empty(SLOTS, np.int64)
            inv0[l0] = np.arange(SLOTS)               # final -> padded-local
            npad_aux = auxcalls * NI
            ad = np.zeros(npad_aux, np.int64)
            ad[:SLOTS] = l0[inv]                      # region pos -> final slot
            cmb = np.zeros(npad_aux, np.int64)
            cmb[:SLOTS] = rq[inv0]                    # final slot -> region pos
            ads.append(wrap_flat(ad))
            cmbs.append(wrap_flat(cmb))

        xg = np.zeros((SLOTS, DIM), np.float32)
        xg[lslot[nds]] = np.asarray(x, np.float32)[nds]
        xT = np.ascontiguousarray(xg.T)

        in_maps.append({
            "xT": xT,
            "widx": wrap_grid(grid),
            "emask": mask,
            "adidx": np.ascontiguousarray(np.concatenate(ads, axis=1)),
            "cmbidx": np.ascontiguousarray(np.concatenate(cmbs, axis=1)),
            "W1": np.asarray(W1, np.float32),
            "W2": np.asarray(W2, np.float32),
            "a1": np.ascontiguousarray(
                np.stack([np.asarray(a_src1, np.float32),
                          np.asarray(a_dst1, np.float32)], axis=1)),
            "a2s": np.tile(np.asarray(a_src2, np.float32)[None, :], (HID, 1)),
            "a2d": np.tile(np.asarray(a_dst2, np.float32)[None, :], (HID, 1)),
            "b1r": np.tile(np.asarray(b1, np.float32)[None, :], (P, 1)),
            "b2r": np.tile(np.asarray(b2, np.float32)[None, :], (P, 1)),
        })

    return meta, in_maps, (core_of, lslot)


# ------------------------------------------------------------------- program

def _build(meta):
    import concourse.bacc as bacc
    import concourse.mybir as mybir
    import concourse.tile as tile
    from concourse.masks import make_identity
    from contextlib import ExitStack

    f32 = mybir.dt.float32
    i16 = mybir.dt.int16
    AF = mybir.ActivationFunctionType
    AX = mybir.AxisListType
    OP = mybir.AluOpType

    DIM, HID, NCLS = meta["DIM"], meta["HID"], meta["NCLS"]
    R, SLOTS, NPAD, QSIZE = meta["R"], meta["SLOTS"], meta["NPAD"], meta["QSIZE"]
    K = DIM // P
    CT, auxcalls = meta["CT"], meta["auxcalls"]
    CQ, tiles = meta["CQ"], meta["tiles"]
    calls_main = meta["calls_main"]

    nc = bacc.Bacc(num_devices=NCORES)
    xT_h = nc.dram_tensor("xT", [DIM, SLOTS], f32, kind="ExternalInput")
    widx_h = nc.dram_tensor("widx", [P, 64 * calls_main], i16, kind="ExternalInput")
    emask_h = nc.dram_tensor("emask", [P, CT], f32, kind="ExternalInput")
    adidx_h = nc.dram_tensor("adidx", [P, 64 * auxcalls * 3], i16, kind="ExternalInput")
    cmbidx_h = nc.dram_tensor("cmbidx", [P, 64 * auxcalls * 3], i16, kind="ExternalInput")
    W1_h = nc.dram_tensor("W1", [DIM, HID], f32, kind="ExternalInput")
    W2_h = nc.dram_tensor("W2", [HID, NCLS], f32, kind="ExternalInput")
    a1_h = nc.dram_tensor("a1", [HID, 2], f32, kind="ExternalInput")
    a2s_h = nc.dram_tensor("a2s", [HID, NCLS], f32, kind="ExternalInput")
    a2d_h = nc.dram_tensor("a2d", [HID, NCLS], f32, kind="ExternalInput")
    b1r_h = nc.dram_tensor("b1r", [P, HID], f32, kind="ExternalInput")
    b2r_h = nc.dram_tensor("b2r", [P, NCLS], f32, kind="ExternalInput")
    out_h = nc.dram_tensor("out", [SLOTS, NCLS], f32, kind="ExternalOutput")

    with tile.TileContext(nc) as tc, ExitStack() as ctx:
        pp = ctx.enter_context(tc.tile_pool(name="persist", bufs=1))
        dram = ctx.enter_context(tc.tile_pool(name="dram", bufs=1, space="DRAM"))
        psp = ctx.enter_context(tc.tile_pool(name="ps", bufs=2, space="PSUM"))
        gp = ctx.enter_context(tc.tile_pool(name="g", bufs=8))
        sp = ctx.enter_context(tc.tile_pool(name="s", bufs=4))
        tp = ctx.enter_context(tc.tile_pool(name="t", bufs=2))
        rq_p = ctx.enter_context(tc.tile_pool(name="rq", bufs=1))

        ident = pp.tile([P, P], f32)
        make_identity(nc, ident[:])
        W2sb = pp.tile([HID, NCLS], f32)
        nc.sync.dma_start(W2sb[:], W2_h[:])
        av1 = pp.tile([HID, 2], f32)
        nc.sync.dma_start(av1[:], a1_h[:])
        b1sb = pp.tile([P, HID], f32)
        nc.sync.dma_start(b1sb[:], b1r_h[:])
        b2sb = pp.tile([P, NCLS], f32)
        nc.sync.dma_start(b2sb[:], b2r_h[:])
        emask = pp.tile([P, CT], f32)
        nc.sync.dma_start(emask[:], emask_h[:])

        # av2 = [W2 @ a_src2, W2 @ a_dst2]  (per-partition over HID)
        av2 = pp.tile([HID, 2], f32)
        for j, ah in enumerate((a2s_h, a2d_h)):
            asb = tp.tile([HID, NCLS], f32, tag="a2")
            nc.sync.dma_start(asb[:], ah[:])
            prod = tp.tile([HID, NCLS], f32, tag="a2p")
            nc.vector.tensor_mul(prod[:], W2sb[:], asb[:])
            nc.vector.reduce_sum(av2[:, j:j + 1], prod[:], axis=AX.X)

        table1 = dram.tile([NPAD, EB], f32)
        table2 = dram.tile([NPAD, EB], f32)
        tstage1 = dram.tile([SLOTS, EB], f32)
        tstage2 = dram.tile([SLOTS, EB], f32)
        pstages = [[dram.tile([SLOTS, EB], f32, name=f"pstage_{l}_{q}")
                    for q in range(3)] for l in range(2)]

        zEB = pp.tile([P, R, EB], f32)
        nc.vector.memset(zEB[:], 0.0)
        h1 = pp.tile([P, R, HID], f32)
        asad1 = pp.tile([P, R, 2], f32)
        h2 = pp.tile([P, R, HID], f32)
        asad2 = pp.tile([P, R, 2], f32)
        accS = pp.tile([P, R], f32)
        accA = pp.tile([P, R, HID], f32)

        def project(h, asad, av):
            """asad[:, r, :] = (h[:, r, :] @ av) via PE transpose + matmul."""
            for r in range(R):
                pt = psp.tile([HID, P], f32, tag="pt")
                nc.tensor.transpose(pt[:], h[:, r, :], ident[:])
                hT = tp.tile([HID, P], f32, tag="hT")
                nc.vector.tensor_copy(hT[:], pt[:])
                pr = psp.tile([P, 2], f32, tag="pr")
                nc.tensor.matmul(pr[:], lhsT=hT[:], rhs=av[:], start=True, stop=True)
                nc.vector.tensor_copy(asad[:, r, :], pr[:])

        def stage_write(stg, asad, h):
            v = stg[:].rearrange("(r p) e -> p r e", p=P)
            nc.sync.dma_start(v[:, :, 0:1], asad[:, :, 0:1])
            nc.sync.dma_start(v[:, :, 1:1 + HID], h[:])
            nc.sync.dma_start(v[:, :, 1 + HID:2 + HID], asad[:, :, 1:2])
            nc.sync.dma_start(v[:, :, 2 + HID:], zEB[:, :, 2 + HID:])

        # ---------------- Phase A: h1, projections, table1
        with tc.tile_pool(name="phA", bufs=4) as pA:
            W1sb = pA.tile([P, K, HID], f32, bufs=1)
            nc.sync.dma_start(W1sb[:], W1_h[:].rearrange("(k p) f -> p k f", p=P))
            xTv = xT_h[:].rearrange("(k p) s -> p k s", p=P)
            for r in range(R):
                xt = pA.tile([P, K, P], f32, tag="xt")
                nc.sync.dma_start(xt[:], xTv[:, :, r * P:(r + 1) * P])
                ps = psp.tile([P, HID], f32, tag="mm1")
                for k in range(K):
                    nc.tensor.matmul(ps[:], lhsT=xt[:, k, :],
                                     rhs=W1sb[:, k, :],
                                     start=(k == 0), stop=(k == K - 1))
                nc.vector.tensor_copy(h1[:, r, :], ps[:])
            project(h1, asad1, av1)
            stage_write(tstage1, asad1, h1)
        nc.gpsimd.collective_compute(
            "AllGather", mybir.AluOpType.bypass,
            replica_groups=[list(range(NCORES))],
            ins=[tstage1.opt()], outs=[table1.opt()])

        # ---------------- edge phase
        def edge_phase(table, tstage, asad, layer):
            nc.vector.memset(accS[:], 0.0)
            nc.vector.memset(accA[:], 0.0)
            colbase = 0
            callbase = 0
            for q in range(NREG):
                ncalls = CQ[q] // TW
                if q == 0:
                    sq, aq = accS, accA

                    def ad_ap(b, w):
                        return asad[:, b, 1:2].to_broadcast([P, w])
                else:
                    sq = rq_p.tile([P, R], f32, tag="sq")
                    aq = rq_p.tile([P, R, HID], f32, tag="aq")
                    nc.vector.memset(sq[:], 0.0)
                    nc.vector.memset(aq[:], 0.0)
                    adg = rq_p.tile([P, auxcalls * TW, EB], f32, tag="adg")
                    adix = rq_p.tile([P, 64 * auxcalls], i16, tag="adix")
                    nc.sync.dma_start(
                        adix[:],
                        adidx_h[:, (q - 1) * 64 * auxcalls:q * 64 * auxcalls])
                    for a in range(auxcalls):
                        nc.gpsimd.dma_gather(
                            adg[:, a * TW:(a + 1) * TW, :], tstage[:],
                            adix[:, a * 64:(a + 1) * 64], NI, NI, EB,
                            single_packet=True)

                    def ad_ap(b, w, adg=adg):
                        return adg[:, b, 1 + HID:2 + HID].to_broadcast([P, w])

                idxsb = rq_p.tile([P, 64 * max(CQ) // TW], i16, tag="idx")
                nc.sync.dma_start(
                    idxsb[:, :64 * ncalls],
                    widx_h[:, 64 * callbase:64 * (callbase + ncalls)])
                for k in range(ncalls):
                    g = gp.tile([P, TW, EB], f32, tag="g")
                    nc.gpsimd.dma_gather(
                        g[:], table[QSIZE * q:QSIZE * (q + 1), :],
                        idxsb[:, k * 64:(k + 1) * 64], NI, NI, EB,
                        single_packet=True)
                    sc = sp.tile([P, TW], f32, tag="sc")
                    for (c0, w, b) in tiles[q][k]:
                        nc.vector.tensor_add(sc[:, c0:c0 + w],
                                             g[:, c0:c0 + w, 0], ad_ap(b, w))
                    scl = sp.tile([P, TW], f32, tag="scl")
                    nc.vector.tensor_scalar_mul(scl[:], sc[:], NEG_SLOPE)
                    nc.vector.tensor_max(sc[:], sc[:], scl[:])
                    nc.vector.tensor_add(
                        sc[:], sc[:],
                        emask[:, colbase + k * TW:colbase + (k + 1) * TW])
                    for (c0, w, b) in tiles[q][k]:
                        stmp = sp.tile([P, 1], f32, tag="st")
                        nc.scalar.activation(sc[:, c0:c0 + w], sc[:, c0:c0 + w],
                                             AF.Exp, accum_out=stmp[:])
                        nc.vector.tensor_add(sq[:, b:b + 1], sq[:, b:b + 1],
                                             stmp[:])
                        wv = sp.tile([P, TW, HID], f32, tag="wv")
                        nc.vector.tensor_mul(
                            wv[:, :w, :], g[:, c0:c0 + w, 1:1 + HID],
                            sc[:, c0:c0 + w].rearrange("p w -> p w ()")
                            .to_broadcast([P, w, HID]))
                        rt = sp.tile([P, HID], f32, tag="rt")
                        nc.vector.reduce_sum(
                            rt[:], wv[:, :w, :].rearrange("p w f -> p f w"),
                            axis=AX.X)
                        nc.vector.tensor_add(aq[:, b, :], aq[:, b, :], rt[:])
                if q > 0:
                    stg = pstages[layer][q - 1]
                    v = stg[:].rearrange("(r p) e -> p r e", p=P)
                    nc.sync.dma_start(v[:, :, 0:1],
                                      sq[:].rearrange("p r -> p r ()"))
                    nc.sync.dma_start(v[:, :, 1:1 + HID], aq[:])
                    nc.sync.dma_start(v[:, :, 1 + HID:], zEB[:, :, 1 + HID:])
                    cmbix = rq_p.tile([P, 64 * auxcalls], i16, tag="cmbix")
                    nc.sync.dma_start(
                        cmbix[:],
                        cmbidx_h[:, (q - 1) * 64 * auxcalls:q * 64 * auxcalls])
                    for a in range(auxcalls):
                        cg = gp.tile([P, TW, EB], f32, tag="g")
                        nc.gpsimd.dma_gather(
                            cg[:], stg[:], cmbix[:, a * 64:(a + 1) * 64],
                            NI, NI, EB, single_packet=True)
                        wc = min(TW, R - a * TW)
                        if wc <= 0:
                            continue
                        nc.vector.tensor_add(accS[:, a * TW:a * TW + wc],
                                             accS[:, a * TW:a * TW + wc],
                                             cg[:, :wc, 0])
                        nc.vector.tensor_add(accA[:, a * TW:a * TW + wc, :],
                                             accA[:, a * TW:a * TW + wc, :],
                                             cg[:, :wc, 1:1 + HID])
                colbase += CQ[q]
                callbase += ncalls

        def normalize(dsttile, bias_relu):
            tmp = tp.tile([P, R], f32, tag="nrm")
            nc.vector.tensor_scalar_add(tmp[:], accS[:], 1e-30)
            rs = tp.tile([P, R], f32, tag="rs")
            nc.vector.reciprocal(rs[:], tmp[:])
            nc.vector.tensor_mul(
                dsttile[:], accA[:],
                rs[:].rearrange("p r -> p r ()").to_broadcast([P, R, HID]))
            if bias_relu:
                nc.vector.tensor_add(
                    dsttile[:], dsttile[:],
                    b1sb[:].rearrange("p f -> p () f").to_broadcast([P, R, HID]))
                nc.scalar.activation(dsttile[:], dsttile[:], AF.Relu)

        # ---------------- layer 1
        edge_phase(table1, tstage1, asad1, 0)
        normalize(h2, True)

        # ---------------- layer 2 tables
        project(h2, asad2, av2)
        stage_write(tstage2, asad2, h2)
        nc.gpsimd.collective_compute(
            "AllGather", mybir.AluOpType.bypass,
            replica_groups=[list(range(NCORES))],
            ins=[tstage2.opt()], outs=[table2.opt()])

        # ---------------- layer 2
        edge_phase(table2, tstage2, asad2, 1)
        vals = pp.tile([P, R, HID], f32)
        normalize(vals, False)

        # ---------------- output: log_softmax(vals @ W2 + b2)
        with tc.tile_pool(name="phE", bufs=1) as pE:
            raw = pE.tile([P, R, NCLS], f32)
            for r in range(R):
                pt = psp.tile([HID, P], f32, tag="pt")
                nc.tensor.transpose(pt[:], vals[:, r, :], ident[:])
                vT = tp.tile([HID, P], f32, tag="hT")
                nc.vector.tensor_copy(vT[:], pt[:])
                po = psp.tile([P, NCLS], f32, tag="po")
                nc.tensor.matmul(po[:], lhsT=vT[:], rhs=W2sb[:],
                                 start=True, stop=True)
                nc.vector.tensor_copy(raw[:, r, :], po[:])
            nc.vector.tensor_add(
                raw[:], raw[:],
                b2sb[:].rearrange("p f -> p () f").to_broadcast([P, R, NCLS]))
            m = tp.tile([P, R], f32, tag="m")
            nc.vector.reduce_max(m[:], raw[:], axis=AX.X)
            nc.vector.tensor_sub(
                raw[:], raw[:],
                m[:].rearrange("p r -> p r ()").to_broadcast([P, R, NCLS]))
            s = tp.tile([P, R], f32, tag="se")
            for r in range(R):
                et = sp.tile([P, NCLS], f32, tag="et")
                nc.scalar.activation(et[:], raw[:, r, :], AF.Exp,
                                     accum_out=s[:, r:r + 1])
            ls = tp.tile([P, R], f32, tag="ls")
            nc.scalar.activation(ls[:], s[:], AF.Ln)
            nc.vector.tensor_sub(
                raw[:], raw[:],
                ls[:].rearrange("p r -> p r ()").to_broadcast([P, R, NCLS]))
            nc.sync.dma_start(
                out_h[:].rearrange("(r p) e -> p r e", p=P), raw[:])

    nc.compile()
    return nc


# -------------------------------------------------------------------- kernel

def kernel(**inputs):
    x = np.asarray(inputs["x"], np.float32)
    meta, in_maps, (core_of, lslot) = _prep(
        x, inputs["edge_index"], inputs["W1"], inputs["a_src1"],
        inputs["a_dst1"], inputs["b1"], inputs["W2"], inputs["a_src2"],
        inputs["a_dst2"], inputs["b2"])
    nc = _build(meta)
    from concourse.bass_utils import run_bass_kernel_spmd
    res = run_bass_kernel_spmd(nc, in_maps, core_ids=list(range(NCORES)))
    N = meta["N"]
    NCLS = meta["NCLS"]
    out = np.empty((N, NCLS), np.float32)
    for c in range(NCORES):
        sel = np.flatnonzero(core_of == c)
        out[sel] = res.results[c]["out"][lslot[sel]]
    return out
